# revision 31
# baseline (speedup 1.0000x reference)
"""Multi-head attention kernel for Trainium2 (8 NeuronCores, data-parallel over batch).

Per-core computation (batch element b):
  Q = xq @ Wq.T + bq ; K = xk @ Wk.T + bk ; V = xv @ Wv.T + bv
  per head h: S_h = Q_h K_h^T * scale ; P_h = softmax(S_h) ; O_h = P_h V_h
  y = concat(O) @ Wo.T + bo

Layout strategy (all matmuls in fp32r):
  - Transpose x and W on PE (fp32r transpose) into feature-major [d, t] tiles.
  - QT/KT computed feature-major [i, t]; V token-major [s, i] with interleaved
    ones columns (65-wide head blocks) so the AV matmul also emits row sums.
  - Scores computed transposed: S^T[s, t] = (K_h^T)^T Q_h^T, exp on ACT with
    scale folded in; AV computed as O^T[d, t] = V_ext^T probs^T with rowsum row.
  - Normalization: reciprocal of rowsum row, broadcast via K=1 ones matmul,
    multiplied during PSUM eviction into outT.
  - y = outT^T WoT token-major, bias added via DRAM-broadcast tile.
"""

from contextlib import ExitStack

import numpy as np

import concourse.bass as bass
import concourse.mybir as mybir
import concourse.tile as tile
from concourse import bacc
from concourse.bass_utils import run_bass_kernel_spmd
from concourse.masks import make_identity

F32 = mybir.dt.float32
F32R = mybir.dt.float32r
ALU = mybir.AluOpType
ACTF = mybir.ActivationFunctionType

B, T, D, H, HD = 8, 1024, 1024, 16, 64
SCALE = HD**-0.5
P = 128
PT = D // P  # 8 partition tiles
HE = HD + 1  # head block width in V_ext (extra ones column)
DE = H * HE  # 1040


_TRB = [2]


def _transpose_matrix(nc, ident, nat_pool, ps_pool, dst_tiles, src_dram, evict):
    """src_dram [1024, 1024] -> dst_tiles[k] f32r [128, 1024] holding src.T.

    dst col range 512*rg..+512 covers src rows 512*rg..+512.
    evict(psum_ap, k, rg) writes the [128, 512] chunk into dst.
    """
    for rg in range(2):
        nats = []
        for j in range(4):
            natt = nat_pool.tile([P, D], F32R, tag="nat")
            nc.sync.dma_start(
                out=natt,
                in_=src_dram[(4 * rg + j) * P : (4 * rg + j + 1) * P, :].bitcast(F32R),
            )
            nats.append(natt)
        for k in range(PT):
            pst = ps_pool.tile([P, 512], F32, tag="tr", bufs=_TRB[0])
            for j in range(4):
                nc.tensor.transpose(
                    pst[:, 128 * j : 128 * (j + 1)].bitcast(F32R),
                    nats[j][:, 128 * k : 128 * (k + 1)],
                    ident,
                )
            evict(pst, k, rg)


def _build(esc_bufs=8, sc_bufs=2, av_bufs=3, bc_bufs=1, nat_bufs=9, tr_bufs=3, projb=2, yb=2, ytr=6, smallb=4, stages=5, heads=H):
    nc = bacc.Bacc(None, target_bir_lowering=False)
    xq_d = nc.dram_tensor("xq", [T, D], F32, kind="ExternalInput")
    xk_d = nc.dram_tensor("xk", [T, D], F32, kind="ExternalInput")
    xv_d = nc.dram_tensor("xv", [T, D], F32, kind="ExternalInput")
    wq_d = nc.dram_tensor("wq", [D, D], F32, kind="ExternalInput")
    wk_d = nc.dram_tensor("wk", [D, D], F32, kind="ExternalInput")
    wv_d = nc.dram_tensor("wv", [D, D], F32, kind="ExternalInput")
    wo_d = nc.dram_tensor("wo", [D, D], F32, kind="ExternalInput")
    bq_d = nc.dram_tensor("bq", [D], F32, kind="ExternalInput")
    bk_d = nc.dram_tensor("bk", [D], F32, kind="ExternalInput")
    bv_d = nc.dram_tensor("bv", [D], F32, kind="ExternalInput")
    bo_d = nc.dram_tensor("bo", [D], F32, kind="ExternalInput")
    y_d = nc.dram_tensor("y", [T, D], F32, kind="ExternalOutput")

    _TRB[0] = tr_bufs
    with tile.TileContext(nc) as tc, ExitStack() as top:
        consts = top.enter_context(tc.tile_pool(name="consts", bufs=1, side="left"))

        # per-partition bias tiles for QT/KT eviction: [128, 8], col k = b[128k:128k+128]
        bqT = consts.tile([P, PT], F32, tag="bqT")
        nc.gpsimd.dma_start(out=bqT, in_=bq_d[:].rearrange("(k p) -> p k", p=P))
        bkT = consts.tile([P, PT], F32, tag="bkT")
        nc.gpsimd.dma_start(out=bkT, in_=bk_d[:].rearrange("(k p) -> p k", p=P))

        bvx = consts.tile([P, DE], F32, tag="bvx")

        # bo broadcast
        bob = consts.tile([P, D], F32, tag="bob")
        nc.gpsimd.dma_start(
            out=bob, in_=bass.AP(tensor=bo_d, offset=0, ap=[[0, P], [1, D]])
        )

        ones_t = consts.tile([1, HD], F32R, tag="ones")
        zeros16 = consts.tile([P, H], F32R, tag="zeros16")
        ident = consts.tile([P, P], F32R, tag="ident")

        # persistent left pools (created lazily in phase order)
        vext_pool = top.enter_context(tc.tile_pool(name="vext", bufs=PT, side="left"))
        vext = [vext_pool.tile([P, DE], F32R, tag="vext", name=f"vext{i}") for i in range(PT)]

        with ExitStack() as proj_ctx:
            wt_pool = proj_ctx.enter_context(
                tc.tile_pool(name="wt", bufs=PT, side="right")
            )
            xt_pool = proj_ctx.enter_context(
                tc.tile_pool(name="xt", bufs=PT, side="right")
            )
            nat_pool = proj_ctx.enter_context(
                tc.tile_pool(name="natp", bufs=nat_bufs, side="right")
            )
            ps_a = proj_ctx.enter_context(
                tc.tile_pool(name="psA", bufs=2, space="PSUM")
            )

            # staging constants built from rotating nat-pool slots (freed naturally)
            stage1 = nat_pool.tile([P, D], F32, tag="nat", name="stage1")
            make_identity(nc, stage1[:, 0:P])
            nc.scalar.copy(ident, stage1[:, 0:P])
            stage2 = nat_pool.tile([P, D], F32, tag="nat", name="stage2")
            nc.gpsimd.dma_start(
                out=stage2[:, 0:D], in_=bass.AP(tensor=bv_d, offset=0, ap=[[0, P], [1, D]])
            )
            nc.vector.memset(bvx, 1.0)
            nc.vector.tensor_copy(
                bvx.rearrange("p (h x) -> p h x", x=HE)[:, :, 0:HD],
                stage2.rearrange("p (h x) -> p h x", x=HD),
            )
            stage3 = nat_pool.tile([P, D], F32, tag="nat", name="stage3")
            nc.vector.memset(stage3[0:1, 0:HD], 1.0)
            nc.vector.tensor_copy(ones_t, stage3[0:1, 0:HD])
            nc.vector.memset(stage3[:, 512 : 512 + H], 0.0)
            nc.vector.tensor_copy(zeros16, stage3[:, 512 : 512 + H])

            def proj_phase(x_dram, w_dram, nat_ctx):
                """Transpose x and W; returns (xt_tiles, wt_tiles)."""
                xts = [xt_pool.tile([P, D], F32R, tag="xt", name=f"xt{i}") for i in range(PT)]
                wts = [wt_pool.tile([P, DE], F32R, tag="wt", name=f"wt{i}") for i in range(PT)]
                def _evx(pst, k, rg):
                    dst = xts[k][:, 512 * rg : 512 * (rg + 1)]
                    if (k + rg) % 2 == 0:
                        nc.vector.tensor_copy(dst, pst[:, :])
                    else:
                        nc.scalar.copy(dst, pst[:, :])

                _transpose_matrix(nc, ident, nat_ctx, ps_a, xts, x_dram, _evx)
                return xts, wts

            # ---------- V phase (first: all heads need V) ----------
            if True:
                xvt, wvt = proj_phase(xv_d, wv_d, nat_pool)
                # gap columns of WvT_ext must be zero
                for k in range(PT):
                    nc.vector.tensor_copy(
                        wvt[k].rearrange("p (h x) -> p h x", x=HE)[:, :, HD:HE],
                        zeros16,
                    )
                # WvT_ext: transpose of wv with columns remapped into 65-blocks
                def _evwv(pst, k, rg):
                    dst = (
                        wvt[k][:, 520 * rg : 520 * (rg + 1)]
                        .rearrange("p (h x) -> p h x", x=HE)[:, :, 0:HD]
                    )
                    src = pst[:, :].rearrange("p (h x) -> p h x", x=HD)
                    if (k + rg) % 2 == 0:
                        nc.scalar.copy(dst, src)
                    else:
                        nc.vector.tensor_copy(dst, src)

                _transpose_matrix(nc, ident, nat_pool, ps_a, wvt, wv_d, _evwv)
                # V_ext[s, i_ext] = xv @ WvT_ext + bv_ext
                for k in range(PT):
                    for c in range(4):
                        pst = ps_a.tile([P, 260], F32, tag="projv")
                        for j in range(PT):
                            nc.tensor.matmul(
                                pst[:, :],
                                xvt[j][:, 128 * k : 128 * (k + 1)],
                                wvt[j][:, 260 * c : 260 * (c + 1)],
                                start=(j == 0),
                                stop=(j == PT - 1),
                            )
                        nc.vector.tensor_tensor(
                            out=vext[k][:, 260 * c : 260 * (c + 1)],
                            in0=pst[:, :],
                            in1=bvx[:, 260 * c : 260 * (c + 1)],
                            op=ALU.add,
                        )

            # ---------- K phase ----------
            kt_pool = top.enter_context(tc.tile_pool(name="kt", bufs=PT, side="left"))
            kt = [kt_pool.tile([P, T], F32R, tag="kt", name=f"kt{i}") for i in range(PT)]
            if True:
                xkt, wkt = proj_phase(xk_d, wk_d, nat_pool)
                def _evwk(pst, k, rg):
                    dst = wkt[k][:, 512 * rg : 512 * (rg + 1)]
                    if (k + rg) % 2 == 0:
                        nc.scalar.copy(dst, pst[:, :])
                    else:
                        nc.vector.tensor_copy(dst, pst[:, :])

                _transpose_matrix(nc, ident, nat_pool, ps_a, wkt, wk_d, _evwk)
                for k in range(PT):
                    for c in range(2):
                        pst = ps_a.tile([P, 512], F32, tag="proj", bufs=projb)
                        for j in range(PT):
                            nc.tensor.matmul(
                                pst[:, :],
                                wkt[j][:, 128 * k : 128 * (k + 1)],
                                xkt[j][:, 512 * c : 512 * (c + 1)],
                                start=(j == 0),
                                stop=(j == PT - 1),
                            )
                        nc.scalar.activation(
                            out=kt[k][:, 512 * c : 512 * (c + 1)],
                            in_=pst[:, :],
                            func=ACTF.Identity,
                            bias=bkT[:, k : k + 1],
                            scale=1.0,
                        )

            # ---------- Q phase ----------
            qt_pool = top.enter_context(tc.tile_pool(name="qt", bufs=PT, side="left"))
            qt = [qt_pool.tile([P, T], F32R, tag="qt", name=f"qt{i}") for i in range(PT)]
            if True:
                xqt, wqt = proj_phase(xq_d, wq_d, nat_pool)
                def _evwq(pst, k, rg):
                    dst = wqt[k][:, 512 * rg : 512 * (rg + 1)]
                    if (k + rg) % 2 == 0:
                        nc.scalar.copy(dst, pst[:, :])
                    else:
                        nc.vector.tensor_copy(dst, pst[:, :])

                _transpose_matrix(nc, ident, nat_pool, ps_a, wqt, wq_d, _evwq)
                for k in range(PT):
                    for c in range(2):
                        pst = ps_a.tile([P, 512], F32, tag="proj", bufs=projb)
                        for j in range(PT):
                            nc.tensor.matmul(
                                pst[:, :],
                                wqt[j][:, 128 * k : 128 * (k + 1)],
                                xqt[j][:, 512 * c : 512 * (c + 1)],
                                start=(j == 0),
                                stop=(j == PT - 1),
                            )
                        nc.scalar.activation(
                            out=qt[k][:, 512 * c : 512 * (c + 1)],
                            in_=pst[:, :],
                            func=ACTF.Identity,
                            bias=bqT[:, k : k + 1],
                            scale=1.0,
                        )

        if stages < 4:
            nc.compile()
            return nc
        # ---------- attention ----------
        outt_pool = top.enter_context(tc.tile_pool(name="outt", bufs=PT, side="left"))
        outt = [outt_pool.tile([P, T], F32R, tag="outt", name=f"outt{i}") for i in range(PT)]
        nat2_pool = top.enter_context(tc.tile_pool(name="nat2", bufs=4, side="right"))
        wo_nats = []
        for j in range(4):
            wnat = nat2_pool.tile([P, D], F32R, tag="nat2", name=f"wo{j}")
            nc.sync.dma_start(
                out=wnat, in_=wo_d[j * P : (j + 1) * P, :].bitcast(F32R)
            )
            wo_nats.append(wnat)
        with (
            tc.tile_pool(name="esc", bufs=esc_bufs, side="right") as esc_pool,
            tc.tile_pool(name="smalls", bufs=smallb, side="right") as smalls,
            tc.tile_pool(name="psB", bufs=2, space="PSUM") as ps_b,
        ):
            for h in range(heads):
                hi, ro = h // 2, 64 * (h % 2)
                escs = []
                for k in range(PT):
                    pst = ps_b.tile([P, T], F32, tag="sc", bufs=sc_bufs)
                    for c in range(2):
                        nc.tensor.matmul(
                            pst[:, 512 * c : 512 * (c + 1)],
                            kt[hi][ro : ro + 64, 128 * k : 128 * (k + 1)],
                            qt[hi][ro : ro + 64, 512 * c : 512 * (c + 1)],
                            start=True,
                            stop=True,
                        )
                    esc_k = esc_pool.tile([P, T], F32R, tag="esc")
                    nc.scalar.activation(
                        out=esc_k, in_=pst[:, :], func=ACTF.Exp, scale=SCALE
                    )
                    escs.append(esc_k)
                for c in range(2):
                    psav = ps_b.tile([HE, 512], F32, tag="av", bufs=av_bufs)
                    for k in range(PT):
                        nc.tensor.matmul(
                            psav[:, :],
                            vext[k][:, HE * h : HE * (h + 1)],
                            escs[k][:, 512 * c : 512 * (c + 1)],
                            start=(k == 0),
                            stop=(k == PT - 1),
                        )
                    rcp = smalls.tile([1, 512], F32R, tag="rcp")
                    with nc.allow_low_precision(reason="softmax reciprocal in f32r"):
                        nc.vector.reciprocal(rcp, psav[HD : HD + 1, :])
                    psbc = ps_b.tile([HD, 512], F32, tag="bc", bufs=bc_bufs)
                    nc.tensor.matmul(psbc[:, :], ones_t, rcp, start=True, stop=True)
                    bcsb = smalls.tile([HD, 512], F32, tag="bcsb")
                    nc.vector.tensor_copy(bcsb, psbc[:, :])
                    nc.vector.tensor_tensor(
                        out=outt[hi][ro : ro + 64, 512 * c : 512 * (c + 1)],
                        in0=psav[0:HD, :],
                        in1=bcsb,
                        op=ALU.mult,
                    )

        if stages < 5:
            nc.compile()
            return nc
        # ---------- output projection ----------
        with (
            tc.tile_pool(name="wt2", bufs=10, side="right") as wt2_pool,
            tc.tile_pool(name="ysb", bufs=3, side="right") as ysb_pool,
            tc.tile_pool(name="psC", bufs=2, space="PSUM") as ps_c,
        ):
            for c in range(2):
                # WoT half: [i-part, j in 512c..512c+512] from wo rows 512c..+512
                if c == 0:
                    nats = wo_nats
                else:
                    nats = []
                    for j in range(4):
                        wnat = nat2_pool.tile([P, D], F32R, tag="nat2", name=f"wo1{j}")
                        nc.sync.dma_start(
                            out=wnat,
                            in_=wo_d[(4 + j) * P : (5 + j) * P, :].bitcast(F32R),
                        )
                        nats.append(wnat)
                wot = []
                for k in range(PT):
                    pst = ps_c.tile([P, 512], F32, tag="tr", bufs=ytr)
                    for j in range(4):
                        nc.tensor.transpose(
                            pst[:, 128 * j : 128 * (j + 1)].bitcast(F32R),
                            nats[j][:, 128 * k : 128 * (k + 1)],
                            ident,
                        )
                    wot_k = wt2_pool.tile([P, 512], F32R, tag="wt2", name=f"wot{c}{k}")
                    if k % 2 == 0:
                        nc.vector.tensor_copy(wot_k, pst[:, :])
                    else:
                        nc.scalar.copy(wot_k, pst[:, :])
                    wot.append(wot_k)
                for m in range(PT):
                    psy = ps_c.tile([P, 512], F32, tag="y", bufs=yb)
                    for k in range(PT):
                        nc.tensor.matmul(
                            psy[:, :],
                            outt[k][:, 128 * m : 128 * (m + 1)],
                            wot[k][:, :],
                            start=(k == 0),
                            stop=(k == PT - 1),
                        )
                    ysb = ysb_pool.tile([P, 512], F32, tag="ysb")
                    nc.vector.tensor_tensor(
                        out=ysb,
                        in0=psy[:, :],
                        in1=bob[:, 512 * c : 512 * (c + 1)],
                        op=ALU.add,
                    )
                    nc.sync.dma_start(
                        out=y_d[128 * m : 128 * (m + 1), 512 * c : 512 * (c + 1)],
                        in_=ysb,
                    )

    nc.compile()
    return nc


_NC_CACHE = None


def _get_nc():
    global _NC_CACHE
    if _NC_CACHE is None:
        _NC_CACHE = _build()
    return _NC_CACHE


def kernel(**inputs) -> np.ndarray:
    query = np.ascontiguousarray(np.asarray(inputs["query"], dtype=np.float32))
    key = np.ascontiguousarray(np.asarray(inputs["key"], dtype=np.float32))
    value = np.ascontiguousarray(np.asarray(inputs["value"], dtype=np.float32))
    wq = np.ascontiguousarray(np.asarray(inputs["Wq"], dtype=np.float32))
    wk = np.ascontiguousarray(np.asarray(inputs["Wk"], dtype=np.float32))
    wv = np.ascontiguousarray(np.asarray(inputs["Wv"], dtype=np.float32))
    wo = np.ascontiguousarray(np.asarray(inputs["Wo"], dtype=np.float32))
    bq = np.ascontiguousarray(np.asarray(inputs["bq"], dtype=np.float32))
    bk = np.ascontiguousarray(np.asarray(inputs["bk"], dtype=np.float32))
    bv = np.ascontiguousarray(np.asarray(inputs["bv"], dtype=np.float32))
    bo = np.ascontiguousarray(np.asarray(inputs["bo"], dtype=np.float32))

    nc = _get_nc()
    in_maps = []
    for b in range(B):
        in_maps.append(
            {
                "xq": query[b],
                "xk": key[b],
                "xv": value[b],
                "wq": wq,
                "wk": wk,
                "wv": wv,
                "wo": wo,
                "bq": bq,
                "bk": bk,
                "bv": bv,
                "bo": bo,
            }
        )
    res = run_bass_kernel_spmd(nc, in_maps, core_ids=list(range(B)))
    return np.stack([res.results[b]["y"] for b in range(B)], axis=0)


# revision 34
# speedup vs baseline: 1.0078x; 1.0078x over previous
"""Multi-head attention kernel for Trainium2 (8 NeuronCores, data-parallel over batch).

Per-core computation (batch element b):
  Q = xq @ Wq.T + bq ; K = xk @ Wk.T + bk ; V = xv @ Wv.T + bv
  per head h: S_h = Q_h K_h^T * scale ; P_h = softmax(S_h) ; O_h = P_h V_h
  y = concat(O) @ Wo.T + bo

Layout strategy (all matmuls in fp32r):
  - Transpose x and W on PE (fp32r transpose) into feature-major [d, t] tiles.
  - QT/KT computed feature-major [i, t]; V token-major [s, i] with interleaved
    ones columns (65-wide head blocks) so the AV matmul also emits row sums.
  - Scores computed transposed: S^T[s, t] = (K_h^T)^T Q_h^T, exp on ACT with
    scale folded in; AV computed as O^T[d, t] = V_ext^T probs^T with rowsum row.
  - Normalization: reciprocal of rowsum row, broadcast via K=1 ones matmul,
    multiplied during PSUM eviction into outT.
  - y = outT^T WoT token-major, bias added via DRAM-broadcast tile.
"""

from contextlib import ExitStack

import numpy as np

import concourse.bass as bass
import concourse.mybir as mybir
import concourse.tile as tile
from concourse import bacc
from concourse.bass_utils import run_bass_kernel_spmd
from concourse.masks import make_identity

F32 = mybir.dt.float32
F32R = mybir.dt.float32r
ALU = mybir.AluOpType
ACTF = mybir.ActivationFunctionType

B, T, D, H, HD = 8, 1024, 1024, 16, 64
SCALE = HD**-0.5
P = 128
PT = D // P  # 8 partition tiles
HE = HD + 1  # head block width in V_ext (extra ones column)
DE = H * HE  # 1040


_TRB = [2]


def _transpose_matrix(nc, ident, nat_pool, ps_pool, dst_tiles, src_dram, evict):
    """src_dram [1024, 1024] -> dst_tiles[k] f32r [128, 1024] holding src.T.

    dst col range 512*rg..+512 covers src rows 512*rg..+512.
    evict(psum_ap, k, rg) writes the [128, 512] chunk into dst.
    """
    for rg in range(2):
        nats = []
        for j in range(4):
            natt = nat_pool.tile([P, D], F32R, tag="nat")
            nc.sync.dma_start(
                out=natt,
                in_=src_dram[(4 * rg + j) * P : (4 * rg + j + 1) * P, :].bitcast(F32R),
            )
            nats.append(natt)
        for k in range(PT):
            pst = ps_pool.tile([P, 512], F32, tag="tr", bufs=_TRB[0])
            for j in range(4):
                nc.tensor.transpose(
                    pst[:, 128 * j : 128 * (j + 1)].bitcast(F32R),
                    nats[j][:, 128 * k : 128 * (k + 1)],
                    ident,
                )
            evict(pst, k, rg)


def _build(esc_bufs=5, sc_bufs=2, av_bufs=3, bc_bufs=1, nat_bufs=9, tr_bufs=3, projb=2, yb=2, ytr=6, smallb=3, stages=5, heads=H):
    nc = bacc.Bacc(None, target_bir_lowering=False)
    xq_d = nc.dram_tensor("xq", [T, D], F32, kind="ExternalInput")
    xk_d = nc.dram_tensor("xk", [T, D], F32, kind="ExternalInput")
    xv_d = nc.dram_tensor("xv", [T, D], F32, kind="ExternalInput")
    wq_d = nc.dram_tensor("wq", [D, D], F32, kind="ExternalInput")
    wk_d = nc.dram_tensor("wk", [D, D], F32, kind="ExternalInput")
    wv_d = nc.dram_tensor("wv", [D, D], F32, kind="ExternalInput")
    wo_d = nc.dram_tensor("wo", [D, D], F32, kind="ExternalInput")
    bq_d = nc.dram_tensor("bq", [D], F32, kind="ExternalInput")
    bk_d = nc.dram_tensor("bk", [D], F32, kind="ExternalInput")
    bv_d = nc.dram_tensor("bv", [D], F32, kind="ExternalInput")
    bo_d = nc.dram_tensor("bo", [D], F32, kind="ExternalInput")
    y_d = nc.dram_tensor("y", [T, D], F32, kind="ExternalOutput")

    _TRB[0] = tr_bufs
    with tile.TileContext(nc) as tc, ExitStack() as top:
        consts = top.enter_context(tc.tile_pool(name="consts", bufs=1, side="left"))

        # per-partition bias tiles for QT/KT eviction: [128, 8], col k = b[128k:128k+128]
        bqT = consts.tile([P, PT], F32, tag="bqT")
        nc.gpsimd.dma_start(out=bqT, in_=bq_d[:].rearrange("(k p) -> p k", p=P))
        bkT = consts.tile([P, PT], F32, tag="bkT")
        nc.gpsimd.dma_start(out=bkT, in_=bk_d[:].rearrange("(k p) -> p k", p=P))

        bvx = consts.tile([P, DE], F32, tag="bvx")

        # bo broadcast
        bob = consts.tile([P, D], F32, tag="bob")
        nc.gpsimd.dma_start(
            out=bob, in_=bass.AP(tensor=bo_d, offset=0, ap=[[0, P], [1, D]])
        )

        ones_t = consts.tile([1, HD], F32R, tag="ones")
        zeros16 = consts.tile([P, H], F32R, tag="zeros16")
        ident = consts.tile([P, P], F32R, tag="ident")

        # persistent left pools (created lazily in phase order)
        vext_pool = top.enter_context(tc.tile_pool(name="vext", bufs=PT, side="left"))
        vext = [vext_pool.tile([P, DE], F32R, tag="vext", name=f"vext{i}") for i in range(PT)]

        with ExitStack() as proj_ctx:
            wt_pool = proj_ctx.enter_context(
                tc.tile_pool(name="wt", bufs=PT, side="right")
            )
            xt_pool = proj_ctx.enter_context(
                tc.tile_pool(name="xt", bufs=PT, side="right")
            )
            nat_pool = proj_ctx.enter_context(
                tc.tile_pool(name="natp", bufs=nat_bufs, side="right")
            )
            ps_a = proj_ctx.enter_context(
                tc.tile_pool(name="psA", bufs=2, space="PSUM")
            )

            # staging constants built from rotating nat-pool slots (freed naturally)
            stage1 = nat_pool.tile([P, D], F32, tag="nat", name="stage1")
            make_identity(nc, stage1[:, 0:P])
            nc.scalar.copy(ident, stage1[:, 0:P])
            stage2 = nat_pool.tile([P, D], F32, tag="nat", name="stage2")
            nc.gpsimd.dma_start(
                out=stage2[:, 0:D], in_=bass.AP(tensor=bv_d, offset=0, ap=[[0, P], [1, D]])
            )
            nc.vector.memset(bvx, 1.0)
            nc.vector.tensor_copy(
                bvx.rearrange("p (h x) -> p h x", x=HE)[:, :, 0:HD],
                stage2.rearrange("p (h x) -> p h x", x=HD),
            )
            stage3 = nat_pool.tile([P, D], F32, tag="nat", name="stage3")
            nc.vector.memset(stage3[0:1, 0:HD], 1.0)
            nc.vector.tensor_copy(ones_t, stage3[0:1, 0:HD])
            nc.vector.memset(stage3[:, 512 : 512 + H], 0.0)
            nc.vector.tensor_copy(zeros16, stage3[:, 512 : 512 + H])

            def proj_phase(x_dram, w_dram, nat_ctx):
                """Transpose x and W; returns (xt_tiles, wt_tiles)."""
                xts = [xt_pool.tile([P, D], F32R, tag="xt", name=f"xt{i}") for i in range(PT)]
                wts = [wt_pool.tile([P, DE], F32R, tag="wt", name=f"wt{i}") for i in range(PT)]
                def _evx(pst, k, rg):
                    dst = xts[k][:, 512 * rg : 512 * (rg + 1)]
                    if (k + rg) % 2 == 0:
                        nc.vector.tensor_copy(dst, pst[:, :])
                    else:
                        nc.scalar.copy(dst, pst[:, :])

                _transpose_matrix(nc, ident, nat_ctx, ps_a, xts, x_dram, _evx)
                return xts, wts

            # ---------- V phase (first: all heads need V) ----------
            if True:
                xvt, wvt = proj_phase(xv_d, wv_d, nat_pool)
                # gap columns of WvT_ext must be zero
                for k in range(PT):
                    nc.vector.tensor_copy(
                        wvt[k].rearrange("p (h x) -> p h x", x=HE)[:, :, HD:HE],
                        zeros16,
                    )
                # WvT_ext: transpose of wv with columns remapped into 65-blocks
                def _evwv(pst, k, rg):
                    dst = (
                        wvt[k][:, 520 * rg : 520 * (rg + 1)]
                        .rearrange("p (h x) -> p h x", x=HE)[:, :, 0:HD]
                    )
                    src = pst[:, :].rearrange("p (h x) -> p h x", x=HD)
                    if (k + rg) % 2 == 0:
                        nc.scalar.copy(dst, src)
                    else:
                        nc.vector.tensor_copy(dst, src)

                _transpose_matrix(nc, ident, nat_pool, ps_a, wvt, wv_d, _evwv)
                # V_ext[s, i_ext] = xv @ WvT_ext + bv_ext
                for k in range(PT):
                    for c in range(4):
                        pst = ps_a.tile([P, 260], F32, tag="projv")
                        for j in range(PT):
                            nc.tensor.matmul(
                                pst[:, :],
                                xvt[j][:, 128 * k : 128 * (k + 1)],
                                wvt[j][:, 260 * c : 260 * (c + 1)],
                                start=(j == 0),
                                stop=(j == PT - 1),
                            )
                        nc.vector.tensor_tensor(
                            out=vext[k][:, 260 * c : 260 * (c + 1)],
                            in0=pst[:, :],
                            in1=bvx[:, 260 * c : 260 * (c + 1)],
                            op=ALU.add,
                        )

            # ---------- K phase ----------
            kt_pool = top.enter_context(tc.tile_pool(name="kt", bufs=PT, side="left"))
            kt = [kt_pool.tile([P, T], F32R, tag="kt", name=f"kt{i}") for i in range(PT)]
            if True:
                xkt, wkt = proj_phase(xk_d, wk_d, nat_pool)
                def _evwk(pst, k, rg):
                    dst = wkt[k][:, 512 * rg : 512 * (rg + 1)]
                    if (k + rg) % 2 == 0:
                        nc.scalar.copy(dst, pst[:, :])
                    else:
                        nc.vector.tensor_copy(dst, pst[:, :])

                _transpose_matrix(nc, ident, nat_pool, ps_a, wkt, wk_d, _evwk)
                for k in range(PT):
                    for c in range(2):
                        pst = ps_a.tile([P, 512], F32, tag="proj", bufs=projb)
                        for j in range(PT):
                            nc.tensor.matmul(
                                pst[:, :],
                                wkt[j][:, 128 * k : 128 * (k + 1)],
                                xkt[j][:, 512 * c : 512 * (c + 1)],
                                start=(j == 0),
                                stop=(j == PT - 1),
                            )
                        nc.scalar.activation(
                            out=kt[k][:, 512 * c : 512 * (c + 1)],
                            in_=pst[:, :],
                            func=ACTF.Identity,
                            bias=bkT[:, k : k + 1],
                            scale=1.0,
                        )

            # ---------- Q phase ----------
            qt_pool = top.enter_context(tc.tile_pool(name="qt", bufs=PT, side="left"))
            qt = [qt_pool.tile([P, T], F32R, tag="qt", name=f"qt{i}") for i in range(PT)]
            if True:
                xqt, wqt = proj_phase(xq_d, wq_d, nat_pool)
                def _evwq(pst, k, rg):
                    dst = wqt[k][:, 512 * rg : 512 * (rg + 1)]
                    if (k + rg) % 2 == 0:
                        nc.scalar.copy(dst, pst[:, :])
                    else:
                        nc.vector.tensor_copy(dst, pst[:, :])

                _transpose_matrix(nc, ident, nat_pool, ps_a, wqt, wq_d, _evwq)
                for k in range(PT):
                    for c in range(2):
                        pst = ps_a.tile([P, 512], F32, tag="proj", bufs=projb)
                        for j in range(PT):
                            nc.tensor.matmul(
                                pst[:, :],
                                wqt[j][:, 128 * k : 128 * (k + 1)],
                                xqt[j][:, 512 * c : 512 * (c + 1)],
                                start=(j == 0),
                                stop=(j == PT - 1),
                            )
                        nc.scalar.activation(
                            out=qt[k][:, 512 * c : 512 * (c + 1)],
                            in_=pst[:, :],
                            func=ACTF.Identity,
                            bias=bqT[:, k : k + 1],
                            scale=1.0,
                        )

        if stages < 4:
            nc.compile()
            return nc
        # ---------- attention ----------
        outt_pool = top.enter_context(tc.tile_pool(name="outt", bufs=PT, side="left"))
        outt = [outt_pool.tile([P, T], F32R, tag="outt", name=f"outt{i}") for i in range(PT)]
        nat2_pool = top.enter_context(tc.tile_pool(name="nat2", bufs=4, side="right"))
        wo_nats = []
        for j in range(4):
            wnat = nat2_pool.tile([P, D], F32R, tag="nat2", name=f"wo{j}")
            nc.sync.dma_start(
                out=wnat, in_=wo_d[j * P : (j + 1) * P, :].bitcast(F32R)
            )
            wo_nats.append(wnat)
        wt2_pool = top.enter_context(tc.tile_pool(name="wt2", bufs=10, side="right"))
        wot0 = []
        with (
            tc.tile_pool(name="esc", bufs=esc_bufs, side="right") as esc_pool,
            tc.tile_pool(name="smalls", bufs=smallb, side="right") as smalls,
            tc.tile_pool(name="psB", bufs=2, space="PSUM") as ps_b,
        ):
            for h in range(heads):
                hi, ro = h // 2, 64 * (h % 2)
                escs = []
                for k in range(PT):
                    pst = ps_b.tile([P, T], F32, tag="sc", bufs=sc_bufs)
                    for c in range(2):
                        nc.tensor.matmul(
                            pst[:, 512 * c : 512 * (c + 1)],
                            kt[hi][ro : ro + 64, 128 * k : 128 * (k + 1)],
                            qt[hi][ro : ro + 64, 512 * c : 512 * (c + 1)],
                            start=True,
                            stop=True,
                        )
                    esc_k = esc_pool.tile([P, T], F32R, tag="esc")
                    nc.scalar.activation(
                        out=esc_k, in_=pst[:, :], func=ACTF.Exp, scale=SCALE
                    )
                    escs.append(esc_k)
                psavs = []
                for c in range(2):
                    psav = ps_b.tile(
                        [HE, 512], F32, tag="av", bufs=av_bufs, name=f"av{h}{c}"
                    )
                    psavs.append(psav)
                for k in range(PT):
                    for c in range(2):
                        nc.tensor.matmul(
                            psavs[c][:, :],
                            vext[k][:, HE * h : HE * (h + 1)],
                            escs[k][:, 512 * c : 512 * (c + 1)],
                            start=(k == 0),
                            stop=(k == PT - 1),
                            skip_group_check=True,
                        )
                if h >= 8:
                    kk = h - 8
                    pstw = ps_b.tile([P, 512], F32, tag="bc", bufs=bc_bufs, name=f"ptw{kk}")
                    for j in range(4):
                        nc.tensor.transpose(
                            pstw[:, 128 * j : 128 * (j + 1)].bitcast(F32R),
                            wo_nats[j][:, 128 * kk : 128 * (kk + 1)],
                            ident,
                        )
                    wot_k = wt2_pool.tile([P, 512], F32R, tag="wt2", name=f"wot0{kk}")
                    nc.vector.tensor_copy(wot_k, pstw[:, :])
                    wot0.append(wot_k)
                for c in range(2):
                    psav = psavs[c]
                    rcp = smalls.tile([1, 512], F32R, tag="rcp")
                    with nc.allow_low_precision(reason="softmax reciprocal in f32r"):
                        nc.vector.reciprocal(rcp, psav[HD : HD + 1, :])
                    psbc = ps_b.tile([HD, 512], F32, tag="bc", bufs=bc_bufs)
                    nc.tensor.matmul(psbc[:, :], ones_t, rcp, start=True, stop=True)
                    bcsb = smalls.tile([HD, 512], F32, tag="bcsb")
                    nc.vector.tensor_copy(bcsb, psbc[:, :])
                    nc.vector.tensor_tensor(
                        out=outt[hi][ro : ro + 64, 512 * c : 512 * (c + 1)],
                        in0=psav[0:HD, :],
                        in1=bcsb,
                        op=ALU.mult,
                    )

        if stages < 5:
            nc.compile()
            return nc
        # ---------- output projection ----------
        with (
            tc.tile_pool(name="ysb", bufs=3, side="right") as ysb_pool,
            tc.tile_pool(name="psC", bufs=2, space="PSUM") as ps_c,
        ):
            for c in range(2):
                # WoT half: [i-part, j in 512c..512c+512] from wo rows 512c..+512
                if c == 0:
                    wot = wot0
                else:
                    nats = []
                    for j in range(4):
                        wnat = nat2_pool.tile([P, D], F32R, tag="nat2", name=f"wo1{j}")
                        nc.sync.dma_start(
                            out=wnat,
                            in_=wo_d[(4 + j) * P : (5 + j) * P, :].bitcast(F32R),
                        )
                        nats.append(wnat)
                    wot = []
                    for k in range(PT):
                        pst = ps_c.tile([P, 512], F32, tag="tr", bufs=ytr)
                        for j in range(4):
                            nc.tensor.transpose(
                                pst[:, 128 * j : 128 * (j + 1)].bitcast(F32R),
                                nats[j][:, 128 * k : 128 * (k + 1)],
                                ident,
                            )
                        wot_k = wt2_pool.tile([P, 512], F32R, tag="wt2", name=f"wot1{k}")
                        if k % 2 == 0:
                            nc.vector.tensor_copy(wot_k, pst[:, :])
                        else:
                            nc.scalar.copy(wot_k, pst[:, :])
                        wot.append(wot_k)
                for m in range(PT):
                    psy = ps_c.tile([P, 512], F32, tag="y", bufs=yb)
                    for k in range(PT):
                        nc.tensor.matmul(
                            psy[:, :],
                            outt[k][:, 128 * m : 128 * (m + 1)],
                            wot[k][:, :],
                            start=(k == 0),
                            stop=(k == PT - 1),
                        )
                    ysb = ysb_pool.tile([P, 512], F32, tag="ysb")
                    nc.vector.tensor_tensor(
                        out=ysb,
                        in0=psy[:, :],
                        in1=bob[:, 512 * c : 512 * (c + 1)],
                        op=ALU.add,
                    )
                    nc.sync.dma_start(
                        out=y_d[128 * m : 128 * (m + 1), 512 * c : 512 * (c + 1)],
                        in_=ysb,
                    )

    nc.compile()
    return nc


_NC_CACHE = None


def _get_nc():
    global _NC_CACHE
    if _NC_CACHE is None:
        _NC_CACHE = _build()
    return _NC_CACHE


def kernel(**inputs) -> np.ndarray:
    query = np.ascontiguousarray(np.asarray(inputs["query"], dtype=np.float32))
    key = np.ascontiguousarray(np.asarray(inputs["key"], dtype=np.float32))
    value = np.ascontiguousarray(np.asarray(inputs["value"], dtype=np.float32))
    wq = np.ascontiguousarray(np.asarray(inputs["Wq"], dtype=np.float32))
    wk = np.ascontiguousarray(np.asarray(inputs["Wk"], dtype=np.float32))
    wv = np.ascontiguousarray(np.asarray(inputs["Wv"], dtype=np.float32))
    wo = np.ascontiguousarray(np.asarray(inputs["Wo"], dtype=np.float32))
    bq = np.ascontiguousarray(np.asarray(inputs["bq"], dtype=np.float32))
    bk = np.ascontiguousarray(np.asarray(inputs["bk"], dtype=np.float32))
    bv = np.ascontiguousarray(np.asarray(inputs["bv"], dtype=np.float32))
    bo = np.ascontiguousarray(np.asarray(inputs["bo"], dtype=np.float32))

    nc = _get_nc()
    in_maps = []
    for b in range(B):
        in_maps.append(
            {
                "xq": query[b],
                "xk": key[b],
                "xv": value[b],
                "wq": wq,
                "wk": wk,
                "wv": wv,
                "wo": wo,
                "bq": bq,
                "bk": bk,
                "bv": bv,
                "bo": bo,
            }
        )
    res = run_bass_kernel_spmd(nc, in_maps, core_ids=list(range(B)))
    return np.stack([res.results[b]["y"] for b in range(B)], axis=0)
